# revision 30
# baseline (speedup 1.0000x reference)
"""Balanced focal NT-Xent loss on 8 TRN2 NeuronCores.

Math (per row i of the 8192x8192 similarity matrix):
  S_i   = sum_j exp(2 * zn_i . zn_j)          (full row sum, incl. diagonal)
  ce_i  = ln(S_i - e^2) - pos_i               (diag term is exactly e^2)
  pos_i = 2 * zn_i . zn_partner(i)
  out   = mean(0.25 * (1 - exp(-ce_i))^2 * ce_i)

Sharding: data-parallel over rows; every core receives the FULL z in two
bf16 layouts, with columns (transposed layout) / rows (row-major layout)
rotated by -core*1024 so that "own rows" are a static slice of the same
program on every core (pure SPMD, no partition-id, no collectives).
"""

import sys

if "/opt/trn_rl_repo" not in sys.path:
    sys.path.insert(0, "/opt/trn_rl_repo")

import numpy as np
import ml_dtypes

import concourse.bass as bass
import concourse.tile as tile
from concourse import bacc, mybir
from concourse.bass_utils import run_bass_kernel_spmd

B = 4096
D = 256
N = 2 * B          # 8192
NCORES = 8
RPC = N // NCORES  # 1024 rows per core
NRT = RPC // 128   # 8 row tiles per core
TEMPERATURE = 0.5
GAMMA = 2.0
ALPHA = 0.25
E2 = float(np.exp(2.0))

BF16 = mybir.dt.bfloat16
F32 = mybir.dt.float32

CG = 2048            # main-loop column group (4 PSUM banks)
NCG = N // CG        # 4
SBLK = 2048          # setup column block
NSB = N // SBLK      # 4
ZR_T_PER_BLK = SBLK // 128  # 16 zr row-tiles per setup block



def _restrict_act_tables(nc):
    """Force Ln and Exp onto the shared natural_log_exp_and_others table set.
    The default per-instruction chooser alternates between the natural_log and
    exp_and_others sets, inserting ~10 ACT_TABLE_LOADs (~1.3us each)."""
    from concourse.hw_specs import get_activation_tables

    tables = get_activation_tables(nc.m.arch)  # cached dict, mutate in place
    keep = "natural_log_exp_and_others"
    if keep in tables:
        for name in tables:
            if name != keep:
                tables[name] = set()


def build_nc():
    nc = bacc.Bacc(None, target_bir_lowering=False)
    _restrict_act_tables(nc)
    # DRAM I/O (bf16 inputs halve the DMA; all stats are re-derived on device)
    zt0 = nc.dram_tensor("zt0", [128, N], BF16, kind="ExternalInput")  # z^T rows 0:128
    zt1 = nc.dram_tensor("zt1", [128, N], BF16, kind="ExternalInput")  # z^T rows 128:256
    zr = nc.dram_tensor("zr", [N, D], BF16, kind="ExternalInput")      # row-major z
    out = nc.dram_tensor("out", [128, NRT], F32, kind="ExternalOutput")

    zts = [zt0, zt1]
    zr_t = zr.rearrange("(t p) d -> t p d", p=128)  # 64 row tiles

    with tile.TileContext(nc) as tc:
        with (
            tc.tile_pool(name="big", bufs=1) as big,
            tc.tile_pool(name="scr", bufs=3) as scr,
            tc.tile_pool(name="rbcp", bufs=2) as rbcp,
            tc.tile_pool(name="stats", bufs=1) as stats,
            tc.tile_pool(name="ps", bufs=2, space="PSUM") as ps,
        ):
            # ---- resident SBUF tensors ----
            # split per half/block so Tile's per-tensor dependency tracking
            # doesn't serialize early readers on late writers
            zt_sb = [
                [
                    big.tile([128, N // 2], BF16, tag=f"zt{c}h{h}",
                             name=f"zt{c}h{h}_sb")
                    for h in range(2)
                ]
                for c in range(2)
            ]
            znt_sb = [
                [
                    big.tile([128, SBLK], BF16, tag=f"znt{c}b{b}",
                             name=f"znt{c}b{b}_sb")
                    for b in range(NSB)
                ]
                for c in range(2)
            ]
            zr_sb = [
                big.tile([128, 32, D], BF16, tag=f"zrh{h}", name=f"zrh{h}_sb")
                for h in range(2)
            ]

            ss_pp = [
                stats.tile([128, 16], F32, tag=f"ss_pp{b}", name=f"ss_pp{b}")
                for b in range(NSB)
            ]
            ln_pp = [
                stats.tile([128, 16], F32, tag=f"ln_pp{b}", name=f"ln_pp{b}")
                for b in range(NSB)
            ]
            r_pp = [
                stats.tile([128, 16], F32, tag=f"r_pp{b}", name=f"r_pp{b}")
                for b in range(NSB)
            ]
            r_ppb = [
                stats.tile([128, 16], BF16, tag=f"r_ppb{b}", name=f"r_ppb{b}")
                for b in range(NSB)
            ]
            dot_pp = stats.tile([128, NRT], F32, tag="dot_pp")
            posf = stats.tile([128, NRT], F32, tag="posf")
            s32 = stats.tile([128, NRT * NCG], F32, tag="s32")
            s8 = stats.tile([128, NRT], F32, tag="s8")
            negE2 = stats.tile([128, 1], F32, tag="negE2")
            nc.vector.memset(negE2, -E2)
            ident = stats.tile([128, 128], BF16, tag="ident")
            from concourse.masks import make_identity
            make_identity(nc, ident)
            def onehot16(t):
                # [16,128] stationary that is 1.0 on row t only: identity
                # column t broadcast along the free dim via a step-0 AP.
                sl = ident[0:16, t:t + 1]
                return bass.AP(tensor=sl.tensor, offset=sl.offset,
                               ap=[sl.ap[0], [0, 128]])
            ce = stats.tile([128, NRT], F32, tag="ce")
            pt = stats.tile([128, NRT], F32, tag="pt")
            u = stats.tile([128, NRT], F32, tag="u")
            outv = stats.tile([128, NRT], F32, tag="outv")

            # ---- input DMAs ----
            # Few big transfers; the two halves interleave so the first half
            # of everything lands early.  sync ring carries the bulk loads;
            # the scalar ring carries the small latency-critical hops below.
            for q in range(2):  # zr first half in 1MB quarters: b0 starts early
                nc.sync.dma_start(
                    out=zr_sb[0][:, q * 16:(q + 1) * 16, :],
                    in_=zr_t[q * 16:(q + 1) * 16, :, :].rearrange("t p d -> p t d"),
                )
            for c in range(2):
                nc.sync.dma_start(out=zt_sb[c][0][:, :], in_=zts[c][:, 0:2 * SBLK])
            nc.sync.dma_start(
                out=zr_sb[1][:, :, :],
                in_=zr_t[32:64, :, :].rearrange("t p d -> p t d"),
            )
            for c in range(2):
                nc.sync.dma_start(out=zt_sb[c][1][:, :], in_=zts[c][:, 2 * SBLK:N])

            # ---- per-block stats: ss -> r (rsqrt via exp(-0.5 ln)) -> rbc -> znt
            rbcs = {}

            def stats_block(b):
                h = b // 2            # which zr/zt half feeds this block
                tb = (b % 2) * 16     # tile offset within the half
                sq16 = scr.tile([128, 16, D], BF16, tag="sq16", name="sq16")
                nc.vector.tensor_mul(
                    sq16, zr_sb[h][:, tb:tb + 16, :], zr_sb[h][:, tb:tb + 16, :]
                )
                nc.vector.tensor_reduce(
                    out=ss_pp[b].rearrange("p (t o) -> p t o", o=1),
                    in_=sq16,
                    axis=mybir.AxisListType.X,
                    op=mybir.AluOpType.add,
                )
                nc.scalar.activation(
                    out=ln_pp[b],
                    in_=ss_pp[b],
                    func=mybir.ActivationFunctionType.Ln,
                )
                nc.scalar.activation(
                    out=r_ppb[b],
                    in_=ln_pp[b],
                    func=mybir.ActivationFunctionType.Exp,
                    scale=-0.5,
                )
                if b in (0, 2):  # fp32 copy only where the pos term needs it
                    nc.scalar.activation(
                        out=r_pp[b],
                        in_=ln_pp[b],
                        func=mybir.ActivationFunctionType.Exp,
                        scale=-0.5,
                    )
                # on-chip row-broadcast of r: PE transpose [128,16] -> [16,128],
                # copy to SBUF, then 16 K=1 matmuls with an all-ones stationary
                # replicate each row across all 128 partitions into PSUM.
                tr_ps = ps.tile([16, 128], BF16, tag="psum", name="tr_ps")
                nc.tensor.transpose(tr_ps, r_ppb[b], ident)
                rT = scr.tile([16, 128], BF16, tag="rT", name="rT")
                nc.vector.tensor_copy(rT, tr_ps)
                rbc_ps = ps.tile([128, SBLK], F32, tag="psum", name="rbc_ps")
                for t in range(16):
                    nc.tensor.matmul(
                        out=rbc_ps[:, t * 128:(t + 1) * 128],
                        lhsT=onehot16(t),
                        rhs=rT,
                        start=True,
                        stop=True,
                    )
                rbc = rbcp.tile([128, SBLK], BF16, tag="rbc", name="rbc")
                nc.vector.tensor_copy(rbc, rbc_ps)
                rbcs[b] = rbc

            def znt_block(b):
                h = b // 2
                xoff = (b % 2) * SBLK
                for c in range(2):
                    nc.vector.tensor_mul(
                        znt_sb[c][b],
                        zt_sb[c][h][:, xoff:xoff + SBLK],
                        rbcs[b],
                    )

            def main_cg(cg):
                for rt in range(NRT):
                    psum = ps.tile([128, CG], F32, tag="psum", name="psum")
                    for c in range(2):
                        lhsT = znt_sb[c][0][:, rt * 128:(rt + 1) * 128]
                        for sgs in range(CG // 512):
                            nc.tensor.matmul(
                                out=psum[:, sgs * 512:(sgs + 1) * 512],
                                lhsT=lhsT,
                                rhs=znt_sb[c][cg][:, sgs * 512:(sgs + 1) * 512],
                                start=(c == 0),
                                stop=(c == 1),
                            )
                    nc.scalar.activation(
                        out=psum,
                        in_=psum,
                        func=mybir.ActivationFunctionType.Exp,
                        scale=2.0,
                        accum_out=s32[:, rt * NCG + cg:rt * NCG + cg + 1],
                    )

            stats_block(0)
            stats_block(1)
            znt_block(0)
            znt_block(1)
            main_cg(0)
            stats_block(2)
            stats_block(3)
            znt_block(2)
            znt_block(3)
            main_cg(1)
            main_cg(2)
            main_cg(3)

            # ---- pos_i = 2 * r_i * r_partner * (z_i . z_partner) ----
            # own rows are zr tiles 0..7, partners are tiles 32..39
            for rt in range(NRT):
                dscr = scr.tile([128, D], F32, tag="dscr")
                nc.vector.tensor_mul(dscr, zr_sb[0][:, rt, :], zr_sb[1][:, rt, :])
                nc.vector.tensor_reduce(
                    out=dot_pp[:, rt:rt + 1],
                    in_=dscr,
                    axis=mybir.AxisListType.X,
                    op=mybir.AluOpType.add,
                )
            nc.vector.tensor_scalar_mul(posf, dot_pp, 2.0)
            nc.vector.tensor_mul(posf, posf, r_pp[0][:, 0:NRT])
            nc.vector.tensor_mul(posf, posf, r_pp[2][:, 0:NRT])

            # ---- epilogue: ce, focal ----
            for rt in range(NRT):
                nc.vector.tensor_reduce(
                    out=s8[:, rt:rt + 1],
                    in_=s32[:, rt * NCG:(rt + 1) * NCG],
                    axis=mybir.AxisListType.X,
                    op=mybir.AluOpType.add,
                )
            nc.scalar.activation(
                out=ce, in_=s8, func=mybir.ActivationFunctionType.Ln, bias=negE2
            )
            nc.vector.tensor_sub(ce, ce, posf)
            nc.scalar.activation(
                out=pt, in_=ce, func=mybir.ActivationFunctionType.Exp, scale=-1.0
            )
            nc.vector.tensor_scalar(
                out=u,
                in0=pt,
                scalar1=-1.0,
                scalar2=1.0,
                op0=mybir.AluOpType.mult,
                op1=mybir.AluOpType.add,
            )
            nc.vector.tensor_mul(u, u, u)
            nc.vector.tensor_mul(u, u, ce)
            nc.vector.tensor_scalar_mul(outv, u, ALPHA)
            nc.sync.dma_start(out=out[:, :], in_=outv)

    nc.finalize()
    return nc


_NC_CACHE = None


def _get_nc():
    global _NC_CACHE
    if _NC_CACHE is None:
        _NC_CACHE = build_nc()
    return _NC_CACHE


def _make_in_maps(zx, zy):
    z = np.concatenate(
        [np.asarray(zx, np.float32), np.asarray(zy, np.float32)], axis=0
    )
    zb = z.astype(ml_dtypes.bfloat16)           # (N, D)
    ztb = np.ascontiguousarray(zb.T)            # (D, N)
    in_maps = []
    for c in range(NCORES):
        sh = c * RPC
        zr_c = np.ascontiguousarray(np.roll(zb, -sh, axis=0))
        zt_c = np.roll(ztb, -sh, axis=1)
        in_maps.append(
            {
                "zt0": np.ascontiguousarray(zt_c[:128]),
                "zt1": np.ascontiguousarray(zt_c[128:]),
                "zr": zr_c,
            }
        )
    return in_maps


def run_device(zx, zy, **kwargs):
    """Run the 8-core kernel; returns (per-row alpha*focal array of shape (N,),
    BassKernelResults)."""
    nc = _get_nc()
    res = run_bass_kernel_spmd(
        nc, _make_in_maps(zx, zy), core_ids=list(range(NCORES)), **kwargs
    )
    focs = []
    for c in range(NCORES):
        o = np.asarray(res.results[c]["out"])  # [128, NRT]
        focs.append(o.T.reshape(-1))           # row = c*RPC + rt*128 + p
    return np.concatenate(focs), res


def kernel(zx, zy):
    foc, _ = run_device(zx, zy)
    return np.float32(np.mean(foc.astype(np.float64)))


if __name__ == "__main__":
    rng = np.random.default_rng(0)
    zx = rng.standard_normal((B, D), dtype=np.float32)
    zy = rng.standard_normal((B, D), dtype=np.float32)
    print(kernel(zx, zy))


# revision 31
# speedup vs baseline: 1.1206x; 1.1206x over previous
"""Balanced focal NT-Xent loss on 8 TRN2 NeuronCores.

Math (per row i of the 8192x8192 similarity matrix):
  S_i   = sum_j exp(2 * zn_i . zn_j)          (full row sum, incl. diagonal)
  ce_i  = ln(S_i - e^2) - pos_i               (diag term is exactly e^2)
  pos_i = 2 * zn_i . zn_partner(i)
  out   = mean(0.25 * (1 - exp(-ce_i))^2 * ce_i)

Sharding: data-parallel over rows; every core receives the FULL z in two
bf16 layouts, with columns (transposed layout) / rows (row-major layout)
rotated by -core*1024 so that "own rows" are a static slice of the same
program on every core (pure SPMD, no partition-id, no collectives).
"""

import sys

if "/opt/trn_rl_repo" not in sys.path:
    sys.path.insert(0, "/opt/trn_rl_repo")

import numpy as np
import ml_dtypes

import concourse.bass as bass
import concourse.tile as tile
from concourse import bacc, mybir
from concourse.bass_utils import run_bass_kernel_spmd

B = 4096
D = 256
N = 2 * B          # 8192
NCORES = 8
RPC = N // NCORES  # 1024 rows per core
NRT = RPC // 128   # 8 row tiles per core
TEMPERATURE = 0.5
GAMMA = 2.0
ALPHA = 0.25
E2 = float(np.exp(2.0))

BF16 = mybir.dt.bfloat16
F32 = mybir.dt.float32

CG = 2048            # main-loop column group (4 PSUM banks)
NCG = N // CG        # 4
SBLK = 2048          # setup column block
NSB = N // SBLK      # 4
ZR_T_PER_BLK = SBLK // 128  # 16 zr row-tiles per setup block



def _restrict_act_tables(nc):
    """Force Ln and Exp onto the shared natural_log_exp_and_others table set.
    The default per-instruction chooser alternates between the natural_log and
    exp_and_others sets, inserting ~10 ACT_TABLE_LOADs (~1.3us each)."""
    from concourse.hw_specs import get_activation_tables

    tables = get_activation_tables(nc.m.arch)  # cached dict, mutate in place
    keep = "natural_log_exp_and_others"
    if keep in tables:
        for name in tables:
            if name != keep:
                tables[name] = set()


def build_nc():
    nc = bacc.Bacc(None, target_bir_lowering=False)
    _restrict_act_tables(nc)
    zt0 = nc.dram_tensor("zt0", [128, N], BF16, kind="ExternalInput")  # z^T rows 0:128
    zt1 = nc.dram_tensor("zt1", [128, N], BF16, kind="ExternalInput")  # z^T rows 128:256
    out = nc.dram_tensor("out", [128, NRT], F32, kind="ExternalOutput")
    zts = [zt0, zt1]

    with tile.TileContext(nc) as tc:
        with (
            tc.tile_pool(name="big", bufs=1) as big,
            tc.tile_pool(name="scr", bufs=3) as scr,
            tc.tile_pool(name="stats", bufs=1) as stats,
            tc.tile_pool(name="ps", bufs=2, space="PSUM") as ps,
        ):
            # per-block tiles (fine-grained deps for Tile's tracker)
            zt_sb = [
                [
                    big.tile([128, SBLK], BF16, tag=f"zt{c}b{b}",
                             name=f"zt{c}b{b}_sb")
                    for b in range(NSB)
                ]
                for c in range(2)
            ]
            znt_sb = [
                [
                    big.tile([128, SBLK], BF16, tag=f"znt{c}b{b}",
                             name=f"znt{c}b{b}_sb")
                    for b in range(NSB)
                ]
                for c in range(2)
            ]
            rbc = [
                big.tile([128, SBLK], BF16, tag=f"rbc{b}", name=f"rbc{b}")
                for b in range(NSB)
            ]

            posd = stats.tile([128, NRT], F32, tag="posd")
            posf = stats.tile([128, NRT], F32, tag="posf")
            s32 = stats.tile([128, NRT * NCG], F32, tag="s32")
            s8 = stats.tile([128, NRT], F32, tag="s8")
            negE2 = stats.tile([128, 1], F32, tag="negE2")
            nc.vector.memset(negE2, -E2)
            onesM = stats.tile([128, 128], BF16, tag="onesM")
            nc.vector.memset(onesM, 1.0)
            ident = stats.tile([128, 128], BF16, tag="ident")
            from concourse.masks import make_identity
            make_identity(nc, ident)
            ce = stats.tile([128, NRT], F32, tag="ce")
            pt = stats.tile([128, NRT], F32, tag="pt")
            u = stats.tile([128, NRT], F32, tag="u")
            outv = stats.tile([128, NRT], F32, tag="outv")

            # ---- per-block: load -> squares -> all-ones matmul (column
            # sums of z^2, replicated over all 128 partitions) -> rsqrt via
            # exp(-0.5 ln) directly in broadcast layout -> zn^T = z^T * rbc
            def stats_block(b):
                sl = slice(b * SBLK, (b + 1) * SBLK)
                for c in range(2):
                    nc.sync.dma_start(out=zt_sb[c][b][:, :], in_=zts[c][:, sl])
                sqs = []
                for c in range(2):
                    sq = scr.tile([128, SBLK], BF16, tag=f"sq{c}", name=f"sq{c}")
                    nc.vector.tensor_mul(sq, zt_sb[c][b], zt_sb[c][b])
                    sqs.append(sq)
                ssbc = ps.tile([128, SBLK], F32, tag="psum", name="ssbc")
                for c in range(2):
                    for s in range(SBLK // 512):
                        nc.tensor.matmul(
                            out=ssbc[:, s * 512:(s + 1) * 512],
                            lhsT=onesM,
                            rhs=sqs[c][:, s * 512:(s + 1) * 512],
                            start=(c == 0),
                            stop=(c == 1),
                        )
                lnt = scr.tile([128, SBLK], F32, tag="lnt", name="lnt")
                nc.scalar.activation(
                    out=lnt, in_=ssbc, func=mybir.ActivationFunctionType.Ln
                )
                nc.scalar.activation(
                    out=rbc[b],
                    in_=lnt,
                    func=mybir.ActivationFunctionType.Exp,
                    scale=-0.5,
                )

            def znt_block(b):
                for c in range(2):
                    nc.vector.tensor_mul(znt_sb[c][b], zt_sb[c][b], rbc[b])

            # ---- main loop: sim row-tile x column-group, fused exp+rowsum.
            # cg==2 covers the partner columns: the positive term is the
            # diagonal of those tiles, extracted pre-exp via identity-mask.
            def main_cg(cg):
                for rt in range(NRT):
                    psum = ps.tile([128, CG], F32, tag="psum", name="psum")
                    for c in range(2):
                        lhsT = znt_sb[c][0][:, rt * 128:(rt + 1) * 128]
                        for s in range(CG // 512):
                            nc.tensor.matmul(
                                out=psum[:, s * 512:(s + 1) * 512],
                                lhsT=lhsT,
                                rhs=znt_sb[c][cg][:, s * 512:(s + 1) * 512],
                                start=(c == 0),
                                stop=(c == 1),
                            )
                    if cg == 2:
                        dg = scr.tile([128, 128], F32, tag="dg", name="dg")
                        nc.vector.tensor_mul(
                            dg, psum[:, rt * 128:(rt + 1) * 128], ident
                        )
                        nc.vector.tensor_reduce(
                            out=posd[:, rt:rt + 1],
                            in_=dg,
                            axis=mybir.AxisListType.X,
                            op=mybir.AluOpType.add,
                        )
                    nc.scalar.activation(
                        out=psum,
                        in_=psum,
                        func=mybir.ActivationFunctionType.Exp,
                        scale=2.0,
                        accum_out=s32[:, rt * NCG + cg:rt * NCG + cg + 1],
                    )

            stats_block(0)
            stats_block(1)
            znt_block(0)
            znt_block(1)
            main_cg(0)
            stats_block(2)
            znt_block(2)
            main_cg(1)
            stats_block(3)
            znt_block(3)
            main_cg(2)
            main_cg(3)

            # ---- epilogue ----
            for rt in range(NRT):
                nc.vector.tensor_reduce(
                    out=s8[:, rt:rt + 1],
                    in_=s32[:, rt * NCG:(rt + 1) * NCG],
                    axis=mybir.AxisListType.X,
                    op=mybir.AluOpType.add,
                )
            nc.scalar.activation(
                out=ce, in_=s8, func=mybir.ActivationFunctionType.Ln, bias=negE2
            )
            nc.vector.tensor_scalar_mul(posf, posd, 2.0)
            nc.vector.tensor_sub(ce, ce, posf)
            nc.scalar.activation(
                out=pt, in_=ce, func=mybir.ActivationFunctionType.Exp, scale=-1.0
            )
            nc.vector.tensor_scalar(
                out=u,
                in0=pt,
                scalar1=-1.0,
                scalar2=1.0,
                op0=mybir.AluOpType.mult,
                op1=mybir.AluOpType.add,
            )
            nc.vector.tensor_mul(u, u, u)
            nc.vector.tensor_mul(u, u, ce)
            nc.vector.tensor_scalar_mul(outv, u, ALPHA)
            nc.sync.dma_start(out=out[:, :], in_=outv)

    nc.finalize()
    return nc


_NC_CACHE = None


def _get_nc():
    global _NC_CACHE
    if _NC_CACHE is None:
        _NC_CACHE = build_nc()
    return _NC_CACHE


def _make_in_maps(zx, zy):
    z = np.concatenate(
        [np.asarray(zx, np.float32), np.asarray(zy, np.float32)], axis=0
    )
    zb = z.astype(ml_dtypes.bfloat16)           # (N, D)
    ztb = np.ascontiguousarray(zb.T)            # (D, N)
    in_maps = []
    for c in range(NCORES):
        sh = c * RPC
        zt_c = np.roll(ztb, -sh, axis=1)
        in_maps.append(
            {
                "zt0": np.ascontiguousarray(zt_c[:128]),
                "zt1": np.ascontiguousarray(zt_c[128:]),
            }
        )
    return in_maps


def run_device(zx, zy, **kwargs):
    """Run the 8-core kernel; returns (per-row alpha*focal array of shape (N,),
    BassKernelResults)."""
    nc = _get_nc()
    res = run_bass_kernel_spmd(
        nc, _make_in_maps(zx, zy), core_ids=list(range(NCORES)), **kwargs
    )
    focs = []
    for c in range(NCORES):
        o = np.asarray(res.results[c]["out"])  # [128, NRT]
        focs.append(o.T.reshape(-1))           # row = c*RPC + rt*128 + p
    return np.concatenate(focs), res


def kernel(zx, zy):
    foc, _ = run_device(zx, zy)
    return np.float32(np.mean(foc.astype(np.float64)))


if __name__ == "__main__":
    rng = np.random.default_rng(0)
    zx = rng.standard_normal((B, D), dtype=np.float32)
    zy = rng.standard_normal((B, D), dtype=np.float32)
    print(kernel(zx, zy))
